# revision 11
# baseline (speedup 1.0000x reference)
"""LSTM regression kernel for 8 Trainium2 NeuronCores (Bass/Tile).

Strategy: 8-way tensor-parallel over the LSTM gate/hidden dimension.
Core j owns hidden slice [j*256, (j+1)*256) and the matching columns of
Wk/Wr (gate-ordered [i | f | o | g] so one sigmoid covers i,f,o).
Per step: z = x_t @ Wk_j + h @ Wr_j + b_j accumulated in PSUM via 18
bf16 matmuls (stationary = transposed activations, moving = weights,
N=1024); gates and cell update on ACT/DVE; the new h slice is
PE-transposed and AllGathered so every core has the full transposed
hidden state for the next step's matmul.
"""
import os
import sys

sys.path.insert(0, "/opt/trn_rl_repo")

import numpy as np
import ml_dtypes

import concourse.bacc as bacc
import concourse.mybir as mybir
from concourse import tile
from concourse.bass_utils import run_bass_kernel_spmd

dt = mybir.dt
bf16 = ml_dtypes.bfloat16

N_CORES = 8
B = 64
F = 256
H = 2048
HS = H // N_CORES          # 256 hidden rows per core
GS = 4 * HS                # 1024 gate columns per core
NKH = H // 128             # 16 hidden contraction chunks
NKX = F // 128             # 2 input contraction chunks

LAST_EXEC_NS = None


def _install_profile_shim():
    """Register the NTFF profiling hook that this image's antenv lacks."""
    import types

    if "antenv.axon_hooks" in sys.modules:
        return
    import antenv
    from trn_agent_boot.trn_boot import _ntff_profile_via_ctypes

    mod = types.ModuleType("antenv.axon_hooks")
    mod._hook = _ntff_profile_via_ctypes("/opt/axon/libaxon_pjrt.so")
    mod.set_axon_ntff_profile_hook = lambda h: setattr(mod, "_hook", h)
    mod.get_axon_ntff_profile_hook = lambda: mod._hook
    sys.modules["antenv.axon_hooks"] = mod
    antenv.axon_hooks = mod


def build_nc(T, bo_val):
    nc = bacc.Bacc(
        "TRN2", target_bir_lowering=False, debug=False, num_devices=N_CORES
    )
    xt = nc.dram_tensor("xt", [T, NKX, 128, B], dt.bfloat16, kind="ExternalInput")
    wr = nc.dram_tensor("wr", [NKH, 128, GS], dt.bfloat16, kind="ExternalInput")
    wk = nc.dram_tensor("wk", [NKX, 128, GS], dt.bfloat16, kind="ExternalInput")
    bt = nc.dram_tensor("bt", [128, GS // 2], dt.float32, kind="ExternalInput")
    wd = nc.dram_tensor("wd", [NKH, 128, 512], dt.bfloat16, kind="ExternalInput")
    bdt = nc.dram_tensor("bdt", [B, 512], dt.float32, kind="ExternalInput")
    wo = nc.dram_tensor("wo", [4, 128, 1], dt.bfloat16, kind="ExternalInput")
    ident = nc.dram_tensor("ident", [128, 128], dt.bfloat16, kind="ExternalInput")
    y = nc.dram_tensor("y", [B, 1], dt.float32, kind="ExternalOutput")

    AF = mybir.ActivationFunctionType
    with tile.TileContext(nc) as tc:
        with (
            tc.tile_pool(name="wpool", bufs=1) as wpool,
            tc.tile_pool(name="spool", bufs=1) as spool,
            tc.tile_pool(name="xpool", bufs=8) as xpool,
            tc.tile_pool(name="zpool", bufs=3) as zpool,
            tc.tile_pool(name="gpool", bufs=3) as gpool,
            tc.tile_pool(name="ppool", bufs=2, space="PSUM") as ppool,
            tc.tile_pool(name="tpool", bufs=4, space="PSUM") as tpool,
            tc.tile_pool(name="dpool", bufs=8, space="DRAM") as dpool,
        ):
            wrt = wpool.tile([128, NKH * GS], dt.bfloat16, tag="wr")
            nc.sync.dma_start(
                wrt[:].rearrange("p (k g) -> p k g", k=NKH),
                wr[:].rearrange("k p g -> p k g"),
            )
            wkt = wpool.tile([128, NKX * GS], dt.bfloat16, tag="wk")
            nc.sync.dma_start(
                wkt[:].rearrange("p (k g) -> p k g", k=NKX),
                wk[:].rearrange("k p g -> p k g"),
            )
            btt = wpool.tile([128, GS // 2], dt.float32, tag="bt")
            nc.sync.dma_start(btt[:], bt[:])
            wdt = wpool.tile([128, NKH * 512], dt.bfloat16, tag="wd")
            nc.sync.dma_start(
                wdt[:].rearrange("p (k g) -> p k g", k=NKH),
                wd[:].rearrange("k p g -> p k g"),
            )
            bdtt = wpool.tile([B, 512], dt.float32, tag="bdt")
            nc.sync.dma_start(bdtt[:], bdt[:])
            wot = wpool.tile([128, 4], dt.bfloat16, tag="wo")
            nc.sync.dma_start(
                wot[:].rearrange("p (k g) -> p k g", g=1),
                wo[:].rearrange("k p g -> p k g"),
            )
            idt = wpool.tile([128, 128], dt.bfloat16, tag="ident")
            nc.sync.dma_start(idt[:], ident[:])

            c_st = spool.tile([128, HS // 2], dt.float32, tag="c")
            nc.gpsimd.memset(c_st[:], 0.0)
            hta = spool.tile([128, H // 2], dt.bfloat16, tag="hta")
            htb = spool.tile([128, H // 2], dt.bfloat16, tag="htb")
            nc.gpsimd.memset(hta[:], 0.0)
            hbufs = [hta, htb]
            hco = spool.tile([128, 128], dt.bfloat16, tag="hco")

            for t in range(T):
                cur = hbufs[t % 2]
                nxt = hbufs[(t + 1) % 2]
                xcur = xpool.tile([128, NKX * B], dt.bfloat16, tag="x")
                nc.sync.dma_start(
                    xcur[:].rearrange("p (k b) -> p k b", k=NKX),
                    xt[t].rearrange("k p b -> p k b"),
                )
                pz = ppool.tile([128, 512], dt.float32, tag="pz")
                nchunks = NKX + NKH
                for idx in range(nchunks):
                    if idx < NKX:
                        stat = xcur[:, idx * B:(idx + 1) * B]
                        wsrc = wkt
                        base = idx * GS
                    else:
                        m = idx - NKX
                        stat = cur[:, m * B:(m + 1) * B]
                        wsrc = wrt
                        base = m * GS
                    nc.tensor.matmul(
                        pz[0:B, :],
                        stat,
                        wsrc[:, base:base + 512],
                        start=(idx == 0),
                        stop=(idx == nchunks - 1),
                        tile_position=(0, 0),
                    )
                    nc.tensor.matmul(
                        pz[B:128, :],
                        stat,
                        wsrc[:, base + 512:base + 1024],
                        start=(idx == 0),
                        stop=(idx == nchunks - 1),
                        tile_position=(0, B),
                    )
                HQ = HS // 2  # 128 gate cols per partition-half
                z = zpool.tile([128, 4 * HQ], dt.float32, tag="z")
                nc.vector.tensor_add(z[:], pz[:], btt[:])
                sg = gpool.tile([128, 3 * HQ], dt.float32, tag="sg")
                nc.scalar.activation(sg[:], z[:, 0:3 * HQ], AF.Sigmoid)
                tg = gpool.tile([128, HQ], dt.float32, tag="tg")
                nc.scalar.activation(tg[:], z[:, 3 * HQ:4 * HQ], AF.Tanh)
                ig = gpool.tile([128, HQ], dt.float32, tag="ig")
                nc.vector.tensor_mul(ig[:], sg[:, 0:HQ], tg[:])
                fc = gpool.tile([128, HQ], dt.float32, tag="fc")
                nc.vector.tensor_mul(fc[:], sg[:, HQ:2 * HQ], c_st[:])
                nc.vector.tensor_add(c_st[:], ig[:], fc[:])
                tch = gpool.tile([128, HQ], dt.float32, tag="tc")
                nc.scalar.activation(tch[:], c_st[:], AF.Tanh)
                hb = gpool.tile([128, HQ], dt.bfloat16, tag="hb")
                nc.vector.tensor_mul(hb[:], sg[:, 2 * HQ:3 * HQ], tch[:])
                din = dpool.tile([128, 128], dt.bfloat16, tag="din")
                for half in range(2):
                    tp = tpool.tile([128, B], dt.bfloat16, tag="tp")
                    nc.tensor.transpose(
                        tp[:],
                        hb[half * B:(half + 1) * B, :],
                        idt[half * B:(half + 1) * B, half * B:(half + 1) * B],
                    )
                    nc.vector.tensor_copy(hco[:, half * B:(half + 1) * B], tp[:])
                nc.sync.dma_start(din[:], hco[:])
                dout = dpool.tile([N_CORES * 128, 128], dt.bfloat16, tag="dout")
                nc.gpsimd.collective_compute(
                    "AllGather",
                    mybir.AluOpType.bypass,
                    replica_groups=[list(range(N_CORES))],
                    ins=[din.opt()],
                    outs=[dout.opt()],
                )
                nxt_v = nxt[:].rearrange("p (r c) -> p r c", r=N_CORES)
                dout_v = dout[:].rearrange("(r p) c -> p r c", r=N_CORES)
                nc.sync.dma_start(nxt_v[:, 0:4], dout_v[:, 0:4])
                nc.sync.dma_start(nxt_v[:, 4:8], dout_v[:, 4:8])

            last = hbufs[T % 2]
            py1 = ppool.tile([B, 512], dt.float32, tag="pz")
            for m in range(NKH):
                nc.tensor.matmul(
                    py1[:],
                    last[:, m * B:(m + 1) * B],
                    wdt[:, m * 512:(m + 1) * 512],
                    start=(m == 0),
                    stop=(m == NKH - 1),
                )
            y1s = zpool.tile([B, 512], dt.float32, tag="y1s")
            nc.vector.tensor_add(y1s[:], py1[:], bdtt[:])
            y1b = zpool.tile([B, 512], dt.bfloat16, tag="y1b")
            nc.scalar.activation(y1b[:], y1s[:], AF.Relu)
            y1t = zpool.tile([128, 4 * B], dt.bfloat16, tag="y1t")
            for q in range(4):
                tq = tpool.tile([128, B], dt.bfloat16, tag="tp")
                nc.tensor.transpose(
                    tq[:], y1b[:, q * 128:(q + 1) * 128], idt[:B, :B]
                )
                nc.vector.tensor_copy(y1t[:, q * B:(q + 1) * B], tq[:])
            pyo = ppool.tile([B, 1], dt.float32, tag="pz")
            for q in range(4):
                nc.tensor.matmul(
                    pyo[:],
                    y1t[:, q * B:(q + 1) * B],
                    wot[:, q:q + 1],
                    start=(q == 0),
                    stop=(q == 3),
                )
            yo = zpool.tile([B, 1], dt.float32, tag="yo")
            nc.scalar.activation(yo[:], pyo[:], AF.Relu, bias=float(bo_val))
            nc.sync.dma_start(y[:], yo[:])
    nc.compile()
    return nc


def kernel(x, Wk, Wr, b, Wd, bd, Wo, bo):
    global LAST_EXEC_NS
    x = np.asarray(x, dtype=np.float32)
    Wk = np.asarray(Wk, dtype=np.float32)
    Wr = np.asarray(Wr, dtype=np.float32)
    b = np.asarray(b, dtype=np.float32)
    Wd = np.asarray(Wd, dtype=np.float32)
    bd = np.asarray(bd, dtype=np.float32)
    Wo = np.asarray(Wo, dtype=np.float32)
    bo = np.asarray(bo, dtype=np.float32)
    T = x.shape[1]

    trace = bool(int(os.environ.get("KERNEL_TRACE", "0")))
    if trace:
        _install_profile_shim()

    nc = build_nc(T, float(bo.reshape(-1)[0]))

    xt_full = np.ascontiguousarray(x.transpose(1, 2, 0)).reshape(
        T, NKX, 128, B
    ).astype(bf16)
    ident_np = np.eye(128, dtype=bf16)
    wd_all = np.ascontiguousarray(Wd.reshape(NKH, 128, 512)).astype(bf16)
    wo_all = np.ascontiguousarray(Wo.reshape(4, 128, 1)).astype(bf16)
    bdt_all = np.tile(bd[None, :], (B, 1)).astype(np.float32)

    gate_perm = [0, 1, 3, 2]  # reference order i,f,g,o -> ours [i f o g]
    in_maps = []
    for j in range(N_CORES):
        js, je = j * HS, (j + 1) * HS
        cols = np.concatenate(
            [
                np.arange(g * H + js + sub * 128, g * H + js + sub * 128 + 128)
                for sub in (0, 1)
                for g in gate_perm
            ]
        )
        wr_j = np.ascontiguousarray(Wr[:, cols]).reshape(NKH, 128, GS).astype(bf16)
        wk_j = np.ascontiguousarray(Wk[:, cols]).reshape(NKX, 128, GS).astype(bf16)
        bt_j = np.concatenate(
            [
                np.tile(b[cols[:512]][None, :], (B, 1)),
                np.tile(b[cols[512:]][None, :], (B, 1)),
            ],
            axis=0,
        ).astype(np.float32)
        in_maps.append(
            {
                "xt": xt_full,
                "wr": wr_j,
                "wk": wk_j,
                "bt": bt_j,
                "wd": wd_all,
                "bdt": bdt_all,
                "wo": wo_all,
                "ident": ident_np,
            }
        )

    res = run_bass_kernel_spmd(
        nc, in_maps, core_ids=list(range(N_CORES)), trace=trace
    )
    LAST_EXEC_NS = res.exec_time_ns
    return res.results[0]["y"].astype(np.float32)


# revision 12
# speedup vs baseline: 113.0803x; 113.0803x over previous
"""LSTM regression kernel for 8 Trainium2 NeuronCores (Bass/Tile).

Strategy: 8-way tensor-parallel over the LSTM gate/hidden dimension.
Core j owns hidden slice [j*256, (j+1)*256) and the matching columns of
Wk/Wr (gate-ordered [i | f | o | g] so one sigmoid covers i,f,o).
Per step: z = x_t @ Wk_j + h @ Wr_j + b_j accumulated in PSUM via 18
bf16 matmuls (stationary = transposed activations, moving = weights,
N=1024); gates and cell update on ACT/DVE; the new h slice is
PE-transposed and AllGathered so every core has the full transposed
hidden state for the next step's matmul.
"""
import os
import sys

sys.path.insert(0, "/opt/trn_rl_repo")

import numpy as np
import ml_dtypes

import concourse.bacc as bacc
import concourse.mybir as mybir
from concourse import tile
from concourse.bass_utils import run_bass_kernel_spmd

dt = mybir.dt
bf16 = ml_dtypes.bfloat16

N_CORES = 8
B = 64
F = 256
H = 2048
HS = H // N_CORES          # 256 hidden rows per core
GS = 4 * HS                # 1024 gate columns per core
NKH = H // 128             # 16 hidden contraction chunks
NKX = F // 128             # 2 input contraction chunks

LAST_EXEC_NS = None


def _install_profile_shim():
    """Register the NTFF profiling hook that this image's antenv lacks."""
    import types

    if "antenv.axon_hooks" in sys.modules:
        return
    import antenv
    from trn_agent_boot.trn_boot import _ntff_profile_via_ctypes

    mod = types.ModuleType("antenv.axon_hooks")
    mod._hook = _ntff_profile_via_ctypes("/opt/axon/libaxon_pjrt.so")
    mod.set_axon_ntff_profile_hook = lambda h: setattr(mod, "_hook", h)
    mod.get_axon_ntff_profile_hook = lambda: mod._hook
    sys.modules["antenv.axon_hooks"] = mod
    antenv.axon_hooks = mod


def build_nc(T, bo_val):
    nc = bacc.Bacc(
        "TRN2", target_bir_lowering=False, debug=False, num_devices=N_CORES
    )
    xt = nc.dram_tensor("xt", [T, NKX, 128, B], dt.bfloat16, kind="ExternalInput")
    wr = nc.dram_tensor("wr", [NKH, 128, GS], dt.bfloat16, kind="ExternalInput")
    wk = nc.dram_tensor("wk", [NKX, 128, GS], dt.bfloat16, kind="ExternalInput")
    bt = nc.dram_tensor("bt", [128, GS // 2], dt.float32, kind="ExternalInput")
    wd = nc.dram_tensor("wd", [NKH, 128, 512], dt.bfloat16, kind="ExternalInput")
    bdt = nc.dram_tensor("bdt", [B, 512], dt.float32, kind="ExternalInput")
    wo = nc.dram_tensor("wo", [4, 128, 1], dt.bfloat16, kind="ExternalInput")
    ident = nc.dram_tensor("ident", [128, 128], dt.bfloat16, kind="ExternalInput")
    y = nc.dram_tensor("y", [B, 1], dt.float32, kind="ExternalOutput")

    AF = mybir.ActivationFunctionType
    with tile.TileContext(nc) as tc:
        with (
            tc.tile_pool(name="wpool", bufs=1) as wpool,
            tc.tile_pool(name="spool", bufs=1) as spool,
            tc.tile_pool(name="xpool", bufs=8) as xpool,
            tc.tile_pool(name="zpool", bufs=2) as zpool,
            tc.tile_pool(name="gpool", bufs=2) as gpool,
            tc.tile_pool(name="ppool", bufs=2, space="PSUM") as ppool,
            tc.tile_pool(name="tpool", bufs=2, space="PSUM") as tpool,
            tc.tile_pool(name="dpool", bufs=4, space="DRAM") as dpool,
        ):
            wrt = wpool.tile([128, NKH * GS], dt.bfloat16, tag="wr")
            nc.sync.dma_start(
                wrt[:].rearrange("p (k g) -> p k g", k=NKH),
                wr[:].rearrange("k p g -> p k g"),
            )
            wkt = wpool.tile([128, NKX * GS], dt.bfloat16, tag="wk")
            nc.sync.dma_start(
                wkt[:].rearrange("p (k g) -> p k g", k=NKX),
                wk[:].rearrange("k p g -> p k g"),
            )
            btt = wpool.tile([128, GS // 2], dt.float32, tag="bt")
            nc.sync.dma_start(btt[:], bt[:])
            wdt = wpool.tile([128, NKH * 512], dt.bfloat16, tag="wd")
            nc.sync.dma_start(
                wdt[:].rearrange("p (k g) -> p k g", k=NKH),
                wd[:].rearrange("k p g -> p k g"),
            )
            bdtt = wpool.tile([B, 512], dt.float32, tag="bdt")
            nc.sync.dma_start(bdtt[:], bdt[:])
            wot = wpool.tile([128, 4], dt.bfloat16, tag="wo")
            nc.sync.dma_start(
                wot[:].rearrange("p (k g) -> p k g", g=1),
                wo[:].rearrange("k p g -> p k g"),
            )
            idt = wpool.tile([128, 128], dt.bfloat16, tag="ident")
            nc.sync.dma_start(idt[:], ident[:])

            c_st = spool.tile([128, HS // 2], dt.float32, tag="c")
            nc.gpsimd.memset(c_st[:], 0.0)
            hta = spool.tile([128, H // 2], dt.bfloat16, tag="hta")
            htb = spool.tile([128, H // 2], dt.bfloat16, tag="htb")
            nc.gpsimd.memset(hta[:], 0.0)
            hbufs = [hta, htb]
            hco = spool.tile([128, 128], dt.bfloat16, tag="hco")

            for t in range(T):
                cur = hbufs[t % 2]
                nxt = hbufs[(t + 1) % 2]
                xcur = xpool.tile([128, NKX * B], dt.bfloat16, tag="x")
                nc.sync.dma_start(
                    xcur[:].rearrange("p (k b) -> p k b", k=NKX),
                    xt[t].rearrange("k p b -> p k b"),
                )
                pz = ppool.tile([128, 512], dt.float32, tag="pz")
                nchunks = NKX + NKH
                for idx in range(nchunks):
                    if idx < NKX:
                        stat = xcur[:, idx * B:(idx + 1) * B]
                        wsrc = wkt
                        base = idx * GS
                    else:
                        m = idx - NKX
                        stat = cur[:, m * B:(m + 1) * B]
                        wsrc = wrt
                        base = m * GS
                    nc.tensor.matmul(
                        pz[0:B, :],
                        stat,
                        wsrc[:, base:base + 512],
                        start=(idx == 0),
                        stop=(idx == nchunks - 1),
                        tile_position=(0, 0),
                    )
                    nc.tensor.matmul(
                        pz[B:128, :],
                        stat,
                        wsrc[:, base + 512:base + 1024],
                        start=(idx == 0),
                        stop=(idx == nchunks - 1),
                        tile_position=(0, B),
                    )
                HQ = HS // 2  # 128 gate cols per partition-half
                z = zpool.tile([128, 4 * HQ], dt.float32, tag="z")
                nc.vector.tensor_add(z[:], pz[:], btt[:])
                sg = gpool.tile([128, 3 * HQ], dt.float32, tag="sg")
                nc.scalar.activation(sg[:], z[:, 0:3 * HQ], AF.Sigmoid)
                tg = gpool.tile([128, HQ], dt.float32, tag="tg")
                nc.scalar.activation(tg[:], z[:, 3 * HQ:4 * HQ], AF.Tanh)
                ig = gpool.tile([128, HQ], dt.float32, tag="ig")
                nc.vector.tensor_mul(ig[:], sg[:, 0:HQ], tg[:])
                fc = gpool.tile([128, HQ], dt.float32, tag="fc")
                nc.vector.tensor_mul(fc[:], sg[:, HQ:2 * HQ], c_st[:])
                nc.vector.tensor_add(c_st[:], ig[:], fc[:])
                tch = gpool.tile([128, HQ], dt.float32, tag="tc")
                nc.scalar.activation(tch[:], c_st[:], AF.Tanh)
                hb = gpool.tile([128, HQ], dt.bfloat16, tag="hb")
                nc.vector.tensor_mul(hb[:], sg[:, 2 * HQ:3 * HQ], tch[:])
                din = dpool.tile([128, 128], dt.bfloat16, tag="din")
                for half in range(2):
                    tp = tpool.tile([128, B], dt.bfloat16, tag="tp")
                    nc.tensor.transpose(
                        tp[:],
                        hb[half * B:(half + 1) * B, :],
                        idt[half * B:(half + 1) * B, half * B:(half + 1) * B],
                    )
                    nc.vector.tensor_copy(hco[:, half * B:(half + 1) * B], tp[:])
                nc.sync.dma_start(din[:], hco[:])
                dout = dpool.tile([N_CORES * 128, 128], dt.bfloat16, tag="dout")
                nc.gpsimd.collective_compute(
                    "AllGather",
                    mybir.AluOpType.bypass,
                    replica_groups=[list(range(N_CORES))],
                    ins=[din.opt()],
                    outs=[dout.opt()],
                )
                nxt_v = nxt[:].rearrange("p (r c) -> p r c", r=N_CORES)
                dout_v = dout[:].rearrange("(r p) c -> p r c", r=N_CORES)
                nc.sync.dma_start(nxt_v[:, 0:4], dout_v[:, 0:4])
                nc.sync.dma_start(nxt_v[:, 4:8], dout_v[:, 4:8])

            last = hbufs[T % 2]
            py1 = ppool.tile([B, 512], dt.float32, tag="pz")
            for m in range(NKH):
                nc.tensor.matmul(
                    py1[:],
                    last[:, m * B:(m + 1) * B],
                    wdt[:, m * 512:(m + 1) * 512],
                    start=(m == 0),
                    stop=(m == NKH - 1),
                )
            y1s = zpool.tile([B, 512], dt.float32, tag="y1s")
            nc.vector.tensor_add(y1s[:], py1[:], bdtt[:])
            y1b = zpool.tile([B, 512], dt.bfloat16, tag="y1b")
            nc.scalar.activation(y1b[:], y1s[:], AF.Relu)
            y1t = zpool.tile([128, 4 * B], dt.bfloat16, tag="y1t")
            for q in range(4):
                tq = tpool.tile([128, B], dt.bfloat16, tag="tp")
                nc.tensor.transpose(
                    tq[:], y1b[:, q * 128:(q + 1) * 128], idt[:B, :B]
                )
                nc.vector.tensor_copy(y1t[:, q * B:(q + 1) * B], tq[:])
            pyo = ppool.tile([B, 1], dt.float32, tag="pz")
            for q in range(4):
                nc.tensor.matmul(
                    pyo[:],
                    y1t[:, q * B:(q + 1) * B],
                    wot[:, q:q + 1],
                    start=(q == 0),
                    stop=(q == 3),
                )
            yo = zpool.tile([B, 1], dt.float32, tag="yo")
            nc.scalar.activation(yo[:], pyo[:], AF.Relu, bias=float(bo_val))
            nc.sync.dma_start(y[:], yo[:])
    nc.compile()
    return nc


def kernel(x, Wk, Wr, b, Wd, bd, Wo, bo):
    global LAST_EXEC_NS
    x = np.asarray(x, dtype=np.float32)
    Wk = np.asarray(Wk, dtype=np.float32)
    Wr = np.asarray(Wr, dtype=np.float32)
    b = np.asarray(b, dtype=np.float32)
    Wd = np.asarray(Wd, dtype=np.float32)
    bd = np.asarray(bd, dtype=np.float32)
    Wo = np.asarray(Wo, dtype=np.float32)
    bo = np.asarray(bo, dtype=np.float32)
    T = x.shape[1]

    trace = bool(int(os.environ.get("KERNEL_TRACE", "0")))
    if trace:
        _install_profile_shim()

    nc = build_nc(T, float(bo.reshape(-1)[0]))

    xt_full = np.ascontiguousarray(x.transpose(1, 2, 0)).reshape(
        T, NKX, 128, B
    ).astype(bf16)
    ident_np = np.eye(128, dtype=bf16)
    wd_all = np.ascontiguousarray(Wd.reshape(NKH, 128, 512)).astype(bf16)
    wo_all = np.ascontiguousarray(Wo.reshape(4, 128, 1)).astype(bf16)
    bdt_all = np.tile(bd[None, :], (B, 1)).astype(np.float32)

    gate_perm = [0, 1, 3, 2]  # reference order i,f,g,o -> ours [i f o g]
    in_maps = []
    for j in range(N_CORES):
        js, je = j * HS, (j + 1) * HS
        cols = np.concatenate(
            [
                np.arange(g * H + js + sub * 128, g * H + js + sub * 128 + 128)
                for sub in (0, 1)
                for g in gate_perm
            ]
        )
        wr_j = np.ascontiguousarray(Wr[:, cols]).reshape(NKH, 128, GS).astype(bf16)
        wk_j = np.ascontiguousarray(Wk[:, cols]).reshape(NKX, 128, GS).astype(bf16)
        bt_j = np.concatenate(
            [
                np.tile(b[cols[:512]][None, :], (B, 1)),
                np.tile(b[cols[512:]][None, :], (B, 1)),
            ],
            axis=0,
        ).astype(np.float32)
        in_maps.append(
            {
                "xt": xt_full,
                "wr": wr_j,
                "wk": wk_j,
                "bt": bt_j,
                "wd": wd_all,
                "bdt": bdt_all,
                "wo": wo_all,
                "ident": ident_np,
            }
        )

    res = run_bass_kernel_spmd(
        nc, in_maps, core_ids=list(range(N_CORES)), trace=trace
    )
    LAST_EXEC_NS = res.exec_time_ns
    return res.results[0]["y"].astype(np.float32)
